# revision 1
# baseline (speedup 1.0000x reference)
"""Efficient Channel Attention kernel for 8 Trainium2 NeuronCores.

Problem (B=4, N=4096, C=1024, H=4, HD=256):
    qkv = x @ Wqkv.T                 -> q,k,v per head, [HD, N] layout
    q,k l2-normalized over N; scores = (q*temp) @ k.T   [HD, HD] per (b,h)
    attn = softmax(scores, -1); out = attn @ v; y = out @ Wproj.T + bproj + x

Sharding: core = (batch b, token-half). All channel contractions are local;
the only cross-core coupling is the token(N)-contracted quantities: the raw
Grams k^T q and the q/k squared norms, AllReduce'd (~1MB) within the core
pair sharing a batch. Device layouts are channel-major (transposed); the
host feeds x^T / W^T slices and transposes the returned y^T back.

SBUF/PSUM pool tags are reused across phases (static pool allocation):
  wgt w0-7   : Wqkv^T qk-cols -> Wqkv^T v-cols -> Wproj^T
  xs  xs0-7  : x^T stream (A1) -> x^T stream (A2) -> B scratch / y + residual
  vo  vo0-8  : v chunks -> out^T chunks
  PSUM pA-pD : q/k accum -> sumsq rows -> v accum -> spm/atp -> proj accum
  PSUM pE,pF : Gram accumulators (2 heads each) -> attn@v accum
"""

import numpy as np

B, N, C, H = 4, 4096, 1024, 4
HD = C // H          # 256
NCORES = 8
NL = N // 2          # 2048 tokens per core
KT = C // 128        # 8 channel k-tiles
NT5 = NL // 512      # 4 token super-tiles
EPS = 1e-12

_CACHE = {}


def _build():
    import concourse.mybir as mybir
    import concourse.tile as tile
    from concourse import bacc
    from concourse.masks import make_identity

    f32 = mybir.dt.float32
    f32r = mybir.dt.float32r
    AX = mybir.AxisListType.X
    ADD = mybir.AluOpType.add
    Exp = mybir.ActivationFunctionType.Exp
    Ident = mybir.ActivationFunctionType.Identity

    nc = bacc.Bacc("TRN2", target_bir_lowering=False, debug=False,
                   num_devices=NCORES)

    xT_d = nc.dram_tensor("xT", [C, NL], f32r, kind="ExternalInput").ap()
    wqkT_d = nc.dram_tensor("wqkT", [C, 2 * C], f32r, kind="ExternalInput").ap()
    wvT_d = nc.dram_tensor("wvT", [C, C], f32r, kind="ExternalInput").ap()
    wpT_d = nc.dram_tensor("wpT", [C, C], f32r, kind="ExternalInput").ap()
    bias_d = nc.dram_tensor("bias", [128, KT], f32, kind="ExternalInput").ap()
    tmpv_d = nc.dram_tensor("tmpv", [128, KT], f32, kind="ExternalInput").ap()
    xrT_d = nc.dram_tensor("xrT", [C, NL], f32r, kind="ExternalInput").ap()
    yT_d = nc.dram_tensor("yT", [C, NL], f32, kind="ExternalOutput").ap()

    with tile.TileContext(nc) as tc:
        with (
            tc.tile_pool(name="const", bufs=1) as constp,
            tc.tile_pool(name="wgt", bufs=1) as wgtp,
            tc.tile_pool(name="xs", bufs=1) as xsp,
            tc.tile_pool(name="vo", bufs=1) as vop,
            tc.tile_pool(name="wrk", bufs=1) as wrk,
            tc.tile_pool(name="ps1", bufs=1, space="PSUM") as ps1,
            tc.tile_pool(name="ps2", bufs=1, space="PSUM") as ps2,
            tc.tile_pool(name="dram", bufs=1, space="DRAM") as dramp,
        ):
            P1 = ["pA", "pB", "pC", "pD"]  # 1-bank rotating psum tags

            # ---------------- constants ----------------
            ident = constp.tile([128, 128], f32, name="ident")
            make_identity(nc, ident[:])
            bias_sb = constp.tile([128, KT], f32, name="bias_sb")
            nc.sync.dma_start(bias_sb[:], bias_d[:])
            tmpv_sb = constp.tile([128, KT], f32, name="tmpv_sb")
            nc.sync.dma_start(tmpv_sb[:], tmpv_d[:])
            ones_sb = constp.tile([128, 1], f32, name="ones_sb")
            nc.vector.memset(ones_sb[:], 1.0)

            # first token super-tile of x^T, loaded ahead of the weights
            xst0 = []
            for kt in range(KT):
                t = xsp.tile([128, 512], f32r, tag=f"xs{kt}", bufs=2,
                             name=f"xa{kt}_0")
                nc.sync.dma_start(t[:], xT_d[kt * 128:(kt + 1) * 128, 0:512])
                xst0.append(t)
            # qk weight chunks, resident through A1
            wqk = []
            for kt in range(KT):
                w = wgtp.tile([128, 2 * C], f32r, tag=f"w{kt}", name=f"wqk{kt}")
                nc.sync.dma_start(w[:], wqkT_d[kt * 128:(kt + 1) * 128, :])
                wqk.append(w)

            # Gram accumulators: stA = heads 0,1 / stB = heads 2,3
            stA = ps2.tile([128, 1024], f32, tag="pE", name="stA")
            stB = ps2.tile([128, 1024], f32, tag="pF", name="stB")

            def st_slice(h, m):
                t = stA if h < 2 else stB
                off = (h % 2) * 512 + m * 256
                return t[:, off:off + 256]

            accq = wrk.tile([128, C], f32, tag="accq", name="accq")
            acck = wrk.tile([128, C], f32, tag="acck", name="acck")

            # ---------------- phase A1: q,k + Grams + sumsq ----------------
            for n5 in range(NT5):
                if n5 == 0:
                    xst = xst0
                else:
                    xst = []
                    for kt in range(KT):
                        t = xsp.tile([128, 512], f32r, tag=f"xs{kt}", bufs=2,
                                     name=f"xa{kt}_{n5}")
                        nc.sync.dma_start(
                            t[:], xT_d[kt * 128:(kt + 1) * 128,
                                       n5 * 512:(n5 + 1) * 512])
                        xst.append(t)
                for s in range(4):
                    tidx = n5 * 4 + s
                    qp0 = ps1.tile([128, 512], f32, tag="pA", name="qp0")
                    qp1 = ps1.tile([128, 512], f32, tag="pB", name="qp1")
                    kp0 = ps1.tile([128, 512], f32, tag="pC", name="kp0")
                    kp1 = ps1.tile([128, 512], f32, tag="pD", name="kp1")
                    for kt in range(KT):
                        lhs = xst[kt][:, s * 128:(s + 1) * 128]
                        fl, ll = (kt == 0), (kt == KT - 1)
                        nc.tensor.matmul(qp0[:], lhs, wqk[kt][:, 0:512],
                                         start=fl, stop=ll)
                        nc.tensor.matmul(qp1[:], lhs, wqk[kt][:, 512:1024],
                                         start=fl, stop=ll)
                        nc.tensor.matmul(kp0[:], lhs, wqk[kt][:, 1024:1536],
                                         start=fl, stop=ll)
                        nc.tensor.matmul(kp1[:], lhs, wqk[kt][:, 1536:2048],
                                         start=fl, stop=ll)
                    qcol = wrk.tile([128, C], f32r, tag="qcol", name="qcol")
                    kcol = wrk.tile([128, C], f32r, tag="kcol", name="kcol")
                    nc.vector.tensor_copy(qcol[:, 0:512], qp0[:])
                    nc.vector.tensor_copy(qcol[:, 512:1024], qp1[:])
                    nc.vector.tensor_copy(kcol[:, 0:512], kp0[:])
                    nc.vector.tensor_copy(kcol[:, 512:1024], kp1[:])
                    sq = wrk.tile([128, C], f32, tag="sq", name="sq")
                    sk = wrk.tile([128, C], f32, tag="sk", name="sk")
                    # square from the SBUF copies so the psum banks free
                    # after a single reader (keeps PE accumulation rolling)
                    nc.scalar.square(sq[:], qcol[:].bitcast(f32))
                    nc.scalar.square(sk[:], kcol[:].bitcast(f32))
                    if tidx == 0:
                        nc.gpsimd.tensor_copy(accq[:], sq[:])
                        nc.gpsimd.tensor_copy(acck[:], sk[:])
                    else:
                        nc.gpsimd.tensor_add(accq[:], accq[:], sq[:])
                        nc.gpsimd.tensor_add(acck[:], acck[:], sk[:])
                    for h in range(H):
                        for m in range(2):
                            nc.tensor.matmul(
                                st_slice(h, m),
                                kcol[:, h * 256 + m * 128: h * 256 + (m + 1) * 128],
                                qcol[:, h * 256:(h + 1) * 256],
                                start=(tidx == 0), stop=(tidx == 15),
                                skip_group_check=True)

            # sumsq rows: [1, 512] ones-matmuls into the freed qk psum slots
            ss_ps = []
            for i, (src, lo) in enumerate([(accq, 0), (accq, 512),
                                           (acck, 0), (acck, 512)]):
                sp = ps1.tile([1, 512], f32, tag=P1[i], name=f"ss{i}")
                nc.tensor.matmul(sp[:], ones_sb[:], src[:, lo:lo + 512],
                                 start=True, stop=True)
                ss_ps.append(sp)

            # SBUF bounces for the collective input (DMA cannot read PSUM);
            # all land in slots whose previous tenants just died.
            stA_sb = wrk.tile([128, 1024], f32, tag="qcol", name="stA_sb")
            stB_sb = wrk.tile([128, 1024], f32, tag="kcol", name="stB_sb")
            nc.vector.tensor_copy(stA_sb[:], stA[:])
            nc.vector.tensor_copy(stB_sb[:], stB[:])
            ss_sb = []
            for i, tg in enumerate(["sq", "sk", "accq", "acck"]):
                sb = wrk.tile([1, 512], f32, tag=tg, name=f"ssb{i}")
                nc.vector.tensor_copy(sb[:], ss_ps[i][:])
                ss_sb.append(sb)

            # ---------------- AllReduce over batch-pairs ----------------
            CCN = 128 * 2048 + 2 * C
            cc_in = dramp.tile([CCN], f32, name="cc_in")
            cc_out = dramp.tile([CCN], f32, name="cc_out")
            nc.sync.dma_start(
                cc_in[0:131072].rearrange("(p f) -> p f", p=128), stA_sb[:])
            nc.sync.dma_start(
                cc_in[131072:262144].rearrange("(p f) -> p f", p=128), stB_sb[:])
            for i in range(4):
                nc.sync.dma_start(
                    cc_in[262144 + i * 512: 262144 + (i + 1) * 512]
                    .rearrange("(a f) -> a f", a=1), ss_sb[i][:])
            nc.gpsimd.collective_compute(
                "AllReduce", ADD,
                replica_groups=[[0, 1], [2, 3], [4, 5], [6, 7]],
                ins=[cc_in.opt()], outs=[cc_out.opt()])
            strA = wrk.tile([128, 1024], f32, tag="qcol", name="strA")
            strB = wrk.tile([128, 1024], f32, tag="kcol", name="strB")
            nc.sync.dma_start(
                strA[:], cc_out[0:131072].rearrange("(p f) -> p f", p=128))
            nc.sync.dma_start(
                strB[:], cc_out[131072:262144].rearrange("(p f) -> p f", p=128))
            ssred = constp.tile([128, 16], f32, name="ssred")
            nc.sync.dma_start(
                ssred[:],
                cc_out[262144:262144 + 2048].rearrange("(j p) -> p j", p=128))

            def str_slice(h, m):
                t = strA if h < 2 else strB
                off = (h % 2) * 512 + m * 256
                return t[:, off:off + 256]

            # ---------------- phase A2: v (overlaps the collective) -------
            wv = []
            for kt in range(KT):
                w = wgtp.tile([128, C], f32r, tag=f"w{kt}", name=f"wv{kt}")
                nc.sync.dma_start(w[:], wvT_d[kt * 128:(kt + 1) * 128, :])
                wv.append(w)
            v_sb = [vop.tile([128, NL], f32r, tag=f"vo{cv}", name=f"v{cv}")
                    for cv in range(8)]
            pcnt = 0
            for pb in range(2):
                xst = []
                for kt in range(KT):
                    ta = xsp.tile([128, 512], f32r, tag=f"xs{kt}", bufs=2,
                                  name=f"xva{kt}_{pb}")
                    tb = xsp.tile([128, 512], f32r, tag=f"xs{kt}", bufs=2,
                                  name=f"xvb{kt}_{pb}")
                    nc.sync.dma_start(
                        ta[:], xT_d[kt * 128:(kt + 1) * 128,
                                    pb * 1024: pb * 1024 + 512])
                    nc.sync.dma_start(
                        tb[:], xT_d[kt * 128:(kt + 1) * 128,
                                    pb * 1024 + 512: pb * 1024 + 1024])
                    xst.append((ta, tb))
                for cv in range(8):
                    va = ps1.tile([128, 512], f32, tag=P1[pcnt % 4], name="vpa")
                    pcnt += 1
                    vb = ps1.tile([128, 512], f32, tag=P1[pcnt % 4], name="vpb")
                    pcnt += 1
                    for kt in range(KT):
                        fl, ll = (kt == 0), (kt == KT - 1)
                        nc.tensor.matmul(va[:],
                                         wv[kt][:, cv * 128:(cv + 1) * 128],
                                         xst[kt][0][:], start=fl, stop=ll)
                        nc.tensor.matmul(vb[:],
                                         wv[kt][:, cv * 128:(cv + 1) * 128],
                                         xst[kt][1][:], start=fl, stop=ll)
                    nc.vector.tensor_copy(
                        v_sb[cv][:, pb * 1024: pb * 1024 + 512], va[:])
                    nc.vector.tensor_copy(
                        v_sb[cv][:, pb * 1024 + 512: pb * 1024 + 1024], vb[:])

            # ---------------- phase B: normalize + softmax + attn@v -------
            # rq = temp/max(sqrt(ssq),eps), rk = 1/max(sqrt(ssk),eps), as
            # per-partition columns [128, 16]: cols 0-7 = rq, 8-15 = rk.
            rqk = constp.tile([128, 16], f32, name="rqk")
            nc.scalar.sqrt(rqk[:], ssred[:])
            nc.vector.tensor_scalar_max(rqk[:], rqk[:], EPS)
            nc.vector.reciprocal(rqk[:], rqk[:])
            nc.vector.tensor_mul(rqk[:, 0:8], rqk[:, 0:8], tmpv_sb[:])

            outT = []
            for h in range(H):
                # Gram^T rows d scaled by rk[d]
                sth = xsp.tile([128, 512], f32, tag="xs4", bufs=2, name="sth")
                for m in range(2):
                    nc.vector.tensor_scalar_mul(
                        sth[:, m * 256:(m + 1) * 256], str_slice(h, m),
                        rqk[:, 8 + 2 * h + m: 9 + 2 * h + m])
                # transpose to S[c, d]
                spm = ps1.tile([128, 512], f32, tag="pA", name="spm")
                for mc in range(2):
                    for md in range(2):
                        nc.tensor.transpose(
                            spm[:, mc * 256 + md * 128: mc * 256 + (md + 1) * 128],
                            sth[:, md * 256 + mc * 128: md * 256 + (mc + 1) * 128],
                            ident[:])
                sft = xsp.tile([128, 512], f32, tag="xs5", bufs=2, name="sft")
                for mc in range(2):
                    nc.vector.tensor_scalar_mul(
                        sft[:, mc * 256:(mc + 1) * 256],
                        spm[:, mc * 256:(mc + 1) * 256],
                        rqk[:, 2 * h + mc: 1 + 2 * h + mc])
                # softmax over d (free axis)
                negmax = wrk.tile([128, 2], f32, tag="negmax", name="negmax")
                rowsum = wrk.tile([128, 2], f32, tag="rowsum", name="rowsum")
                recip = wrk.tile([128, 2], f32, tag="recip", name="recip")
                esb = xsp.tile([128, 512], f32, tag="xs6", bufs=2, name="esb")
                for mc in range(2):
                    nc.vector.reduce_max(negmax[:, mc:mc + 1],
                                         sft[:, mc * 256:(mc + 1) * 256],
                                         axis=AX, negate=True)
                    nc.scalar.activation(esb[:, mc * 256:(mc + 1) * 256],
                                         sft[:, mc * 256:(mc + 1) * 256],
                                         Exp, bias=negmax[:, mc:mc + 1],
                                         accum_out=rowsum[:, mc:mc + 1])
                nc.vector.reciprocal(recip[:], rowsum[:])
                # attn^T (columns d on partitions)
                atp = ps1.tile([128, 512], f32, tag="pB", name="atp")
                for md in range(2):
                    for mc in range(2):
                        nc.tensor.transpose(
                            atp[:, md * 256 + mc * 128: md * 256 + (mc + 1) * 128],
                            esb[:, mc * 256 + md * 128: mc * 256 + (md + 1) * 128],
                            ident[:])
                atn = xsp.tile([128, 512], f32r, tag="xs7", bufs=2, name="atn")
                nc.vector.tensor_copy(atn[:], atp[:])
                # out^T[c,:] = sum_d attn^T[d,c] v[d,:], row-scaled by 1/rowsum
                op2s = {}
                for mc in range(2):
                    for nfh in range(2):
                        op2 = ps2.tile([128, 1024], f32,
                                       tag=("pE" if nfh == 0 else "pF"),
                                       name="op2")
                        op2s[(mc, nfh)] = op2
                        for md in range(2):
                            for n2 in range(2):
                                nc.tensor.matmul(
                                    op2[:, n2 * 512:(n2 + 1) * 512],
                                    atn[:, md * 256 + mc * 128: md * 256 + (mc + 1) * 128],
                                    v_sb[2 * h + md][:, nfh * 1024 + n2 * 512:
                                                     nfh * 1024 + (n2 + 1) * 512],
                                    start=(md == 0), stop=(md == 1))
                for mc in range(2):
                    i = 2 * h + mc
                    ot = vop.tile([128, NL], f32r, tag=f"vo{(i + 8) % 9}",
                                  name=f"ot{i}")
                    outT.append(ot)
                    for nfh in range(2):
                        nc.vector.tensor_scalar_mul(
                            ot[:, nfh * 1024:(nfh + 1) * 1024],
                            op2s[(mc, nfh)][:], recip[:, mc:mc + 1])

            # ---------------- phase C: projection + bias + residual -------
            wp = []
            for kt in range(KT):
                w = wgtp.tile([128, C], f32r, tag=f"w{kt}", name=f"wp{kt}")
                nc.sync.dma_start(w[:], wpT_d[kt * 128:(kt + 1) * 128, :])
                wp.append(w)
            for j in range(KT):
                pq = []
                for q in range(4):
                    p = ps1.tile([128, 512], f32, tag=P1[q], name=f"pp{q}")
                    pq.append(p)
                for kt in range(KT):
                    # proj input channel chunk kt = (jj=kt//2, d-half=kt%2);
                    # column block q is head q; tokens subsampled jj::4
                    for q in range(4):
                        nc.tensor.matmul(
                            pq[q][:],
                            wp[kt][:, j * 128:(j + 1) * 128],
                            outT[2 * q + kt % 2][:, (kt // 2)::4],
                            start=(kt == 0), stop=(kt == KT - 1))
                for q in range(4):
                    xr = xsp.tile([128, 512], f32r, tag=f"xs{4 + q}", bufs=2,
                                  name=f"xr{j}_{q}")
                    nc.sync.dma_start(
                        xr[:], xrT_d[j * 128:(j + 1) * 128,
                                     q * 512:(q + 1) * 512])
                    yq = xsp.tile([128, 512], f32, tag=f"xs{q}", bufs=2,
                                  name=f"yq{j}_{q}")
                    nc.scalar.activation(yq[:], pq[q][:], Ident,
                                         bias=bias_sb[:, j:j + 1])
                    nc.vector.tensor_add(yq[:], yq[:], xr[:].bitcast(f32))
                    nc.sync.dma_start(
                        yT_d[j * 128:(j + 1) * 128, q * 512:(q + 1) * 512],
                        yq[:])

    nc.compile()
    return nc


def _get_nc():
    if "nc" not in _CACHE:
        _CACHE["nc"] = _build()
    return _CACHE["nc"]


def _make_in_maps(x, Wqkv, Wproj, bproj, temperature):
    x = np.ascontiguousarray(np.asarray(x, dtype=np.float32))
    Wqkv = np.asarray(Wqkv, dtype=np.float32)
    Wproj = np.asarray(Wproj, dtype=np.float32)
    bproj = np.asarray(bproj, dtype=np.float32).reshape(C)
    temp = np.asarray(temperature, dtype=np.float32).reshape(H)

    WqkvT = np.ascontiguousarray(Wqkv.T)          # [C, 3C]
    wqkT = np.ascontiguousarray(WqkvT[:, :2 * C])
    wvT = np.ascontiguousarray(WqkvT[:, 2 * C:])
    wpT = np.ascontiguousarray(Wproj.T)
    bias2d = np.ascontiguousarray(bproj.reshape(KT, 128).T)
    tmpv2d = np.ascontiguousarray(np.repeat(temp, HD).reshape(KT, 128).T)

    in_maps = []
    for core in range(NCORES):
        b, half = core // 2, core % 2
        xT = np.ascontiguousarray(x[b, half * NL:(half + 1) * NL, :].T)
        rows = _out_rows(half)
        xrT = np.ascontiguousarray(x[b, rows, :].T)
        in_maps.append(dict(xT=xT, xrT=xrT, wqkT=wqkT, wvT=wvT, wpT=wpT,
                            bias=bias2d, tmpv=tmpv2d))
    return in_maps


def _out_rows(half):
    # torch transpose+reshape scramble: this core's y rows
    return np.concatenate(
        [h * 1024 + half * 512 + np.arange(512) for h in range(H)])


def _run(in_maps, trace=False, **kw):
    from concourse.bass_utils import run_bass_kernel_spmd

    nc = _get_nc()
    return run_bass_kernel_spmd(nc, in_maps, core_ids=list(range(NCORES)),
                                trace=trace, **kw)


def kernel(x, Wqkv, Wproj, bproj, temperature):
    res = _run(_make_in_maps(x, Wqkv, Wproj, bproj, temperature))
    y = np.empty((B, N, C), dtype=np.float32)
    for core in range(NCORES):
        b, half = core // 2, core % 2
        y[b, _out_rows(half), :] = res.results[core]["yT"].T
    return y



# revision 5
# speedup vs baseline: 1.6825x; 1.6825x over previous
"""Efficient Channel Attention kernel for 8 Trainium2 NeuronCores.

Problem (B=4, N=4096, C=1024, H=4, HD=256):
    qkv = x @ Wqkv.T                 -> q,k,v per head, [HD, N] layout
    q,k l2-normalized over N; scores = (q*temp) @ k.T   [HD, HD] per (b,h)
    attn = softmax(scores, -1); out = attn @ v; y = out @ Wproj.T + bproj + x

Sharding: core = (batch b, token-half). All channel contractions are local;
the only cross-core coupling is the token(N)-contracted quantities: the raw
Grams k^T q and the q/k squared norms, AllReduce'd (bf16, ~0.5MB) within
the core pair sharing a batch.

All large GEMMs (qkv, v, Gram, attn@v, proj) run in fp8e4m3 with the
DoubleRow perf mode (2 k-slabs of 128 per instruction). Scales are chosen
so fp8 operands sit in the format's sweet spot and cancel exactly through
the l2 normalization / softmax math:
    x8 = fp8(x); w8 = fp8(32*W)  ->  q,k,v come out 32x
    v8 = fp8(0.5 * v_psum) = 16*v ; atn8 = fp8(64 * exp)
    out8 = fp8(op2 / (16*rowsum)) = 64*out ; proj psum = 2048*y -> scale 1/2048

Tokens are processed in "residue-grouped" order g = (n%4)*512 + n//4 (host
permutes x once). A1/Gram/sumsq are order-invariant; v and out^T then come
out grouped so the proj GEMM's moving operand is fully contiguous and the
torch transpose+reshape scramble costs nothing.
"""

import numpy as np

B, N, C, H = 4, 4096, 1024, 4
HD = C // H          # 256
NCORES = 8
NL = N // 2          # 2048 tokens per core
T4 = C // 256        # 4 channel super-chunks (2 slabs of 128 each)
EPS = 1e-12

_CACHE = {}


def _build():
    import concourse.mybir as mybir
    import concourse.tile as tile
    from concourse import bacc
    from concourse.masks import make_identity

    f32 = mybir.dt.float32
    f32r = mybir.dt.float32r
    bf16 = mybir.dt.bfloat16
    f8 = mybir.dt.float8e4
    DR = mybir.MatmulPerfMode.DoubleRow
    AX = mybir.AxisListType.X
    ADD = mybir.AluOpType.add
    Exp = mybir.ActivationFunctionType.Exp
    Ident = mybir.ActivationFunctionType.Identity

    nc = bacc.Bacc("TRN2", target_bir_lowering=False, debug=False,
                   num_devices=NCORES)

    xs_d = nc.dram_tensor("xs", [T4, 128, 2, NL], f8, kind="ExternalInput").ap()
    wqk_d = nc.dram_tensor("wqk", [T4, 128, 2, 2 * C], f8,
                           kind="ExternalInput").ap()
    wv_d = nc.dram_tensor("wv", [T4, 128, 2, C], f8, kind="ExternalInput").ap()
    wp_d = nc.dram_tensor("wp", [T4, 128, 2, C], f8, kind="ExternalInput").ap()
    bias_d = nc.dram_tensor("bias", [128, 8], f32, kind="ExternalInput").ap()
    tmpv_d = nc.dram_tensor("tmpv", [128, 8], f32, kind="ExternalInput").ap()
    xrT_d = nc.dram_tensor("xrT", [C, NL], f32, kind="ExternalInput").ap()
    yT_d = nc.dram_tensor("yT", [C, NL], f32, kind="ExternalOutput").ap()

    with tile.TileContext(nc) as tc:
        with (
            tc.tile_pool(name="const", bufs=1) as constp,
            tc.tile_pool(name="wgt", bufs=1) as wgtp,
            tc.tile_pool(name="xst", bufs=1) as xsp,
            tc.tile_pool(name="xall", bufs=1) as xap,
            tc.tile_pool(name="qk", bufs=1) as qkp,
            tc.tile_pool(name="sums", bufs=1) as smp,
            tc.tile_pool(name="vo", bufs=1) as vop,
            tc.tile_pool(name="str", bufs=1) as strp,
            tc.tile_pool(name="bph", bufs=1) as bp,
            tc.tile_pool(name="cph", bufs=1) as cp,
            tc.tile_pool(name="ps1", bufs=1, space="PSUM") as ps1,
            tc.tile_pool(name="ps2", bufs=1, space="PSUM") as ps2,
            tc.tile_pool(name="dram", bufs=1, space="DRAM") as dramp,
        ):
            P1 = ["pA", "pB", "pC", "pD"]  # 1-bank rotating psum tags

            # ---------------- constants ----------------
            ident = constp.tile([128, 128], f32, name="ident")
            make_identity(nc, ident[:])
            bias_sb = constp.tile([128, 8], f32, name="bias_sb")
            nc.sync.dma_start(bias_sb[:], bias_d[:])
            tmpv_sb = constp.tile([128, 8], f32, name="tmpv_sb")
            nc.sync.dma_start(tmpv_sb[:], tmpv_d[:])
            ones_sb = constp.tile([128, 1], bf16, name="ones_sb")
            nc.vector.memset(ones_sb[:], 1.0)

            # qk weights + first x super-tile ahead of everything else
            wqkq, wqkk = [], []
            xst0 = []
            for t in range(T4):
                wq = wgtp.tile([128, 2, C], f8, tag=f"wq{t}", name=f"wqkq{t}")
                nc.sync.dma_start(wq[:], wqk_d[t, :, :, 0:C])
                wqkq.append(wq)
                wk = wgtp.tile([128, 2, C], f8, tag=f"wk{t}", name=f"wqkk{t}")
                nc.sync.dma_start(wk[:], wqk_d[t, :, :, C:2 * C])
                wqkk.append(wk)
                xt = xsp.tile([128, 2, 512], f8, tag=f"xs{t}", bufs=2,
                              name=f"xa{t}_0")
                nc.sync.dma_start(xt[:], xs_d[t, :, :, 0:512])
                xst0.append(xt)
            # full x resident for phase A2 (v)
            xall = []
            for t in range(T4):
                xt = xap.tile([128, 2, NL], f8, tag=f"xf{t}", name=f"xall{t}")
                nc.sync.dma_start(xt[:], xs_d[t])
                xall.append(xt)

            # Gram accumulators: stA = heads 0,1 / stB = heads 2,3
            stA = ps2.tile([128, 1024], f32, tag="pE", name="stA")
            stB = ps2.tile([128, 1024], f32, tag="pF", name="stB")

            def st_slice(h, m):
                t = stA if h < 2 else stB
                off = (h % 2) * 512 + m * 256
                return t[:, off:off + 256]

            accq = smp.tile([128, C], f32, tag="accq", name="accq")
            acck = smp.tile([128, C], f32, tag="acck", name="acck")

            # ---------------- phase A1: q,k + Grams + sumsq ----------------
            qc8 = kc8 = None
            xst = xst0
            for s in range(16):
                n5, sub = s // 4, s % 4
                if n5 > 0 and sub == 0:
                    xst = []
                    for t in range(T4):
                        xt = xsp.tile([128, 2, 512], f8, tag=f"xs{t}", bufs=2,
                                      name=f"xa{t}_{n5}")
                        nc.sync.dma_start(
                            xt[:], xs_d[t, :, :, n5 * 512:(n5 + 1) * 512])
                        xst.append(xt)
                h2 = s % 2
                if h2 == 0:
                    qc8 = qkp.tile([128, 2, C], f8, tag="qc8", bufs=2,
                                   name=f"qc8_{s // 2}")
                    kc8 = qkp.tile([128, 2, C], f8, tag="kc8", bufs=2,
                                   name=f"kc8_{s // 2}")
                qp0 = ps1.tile([128, 512], f32, tag="pA", name="qp0")
                qp1 = ps1.tile([128, 512], f32, tag="pB", name="qp1")
                kp0 = ps1.tile([128, 512], f32, tag="pC", name="kp0")
                kp1 = ps1.tile([128, 512], f32, tag="pD", name="kp1")
                for t in range(T4):
                    lhs = xst[t][:, :, sub * 128:(sub + 1) * 128]
                    fl, ll = (t == 0), (t == T4 - 1)
                    nc.tensor.matmul(qp0[:], lhs, wqkq[t][:, :, 0:512],
                                     start=fl, stop=ll, perf_mode=DR)
                    nc.tensor.matmul(qp1[:], lhs, wqkq[t][:, :, 512:1024],
                                     start=fl, stop=ll, perf_mode=DR)
                    nc.tensor.matmul(kp0[:], lhs, wqkk[t][:, :, 0:512],
                                     start=fl, stop=ll, perf_mode=DR)
                    nc.tensor.matmul(kp1[:], lhs, wqkk[t][:, :, 512:1024],
                                     start=fl, stop=ll, perf_mode=DR)
                nc.vector.tensor_copy(qc8[:, h2:h2 + 1, 0:512], qp0[:])
                nc.vector.tensor_copy(qc8[:, h2:h2 + 1, 512:1024], qp1[:])
                nc.vector.tensor_copy(kc8[:, h2:h2 + 1, 0:512], kp0[:])
                nc.vector.tensor_copy(kc8[:, h2:h2 + 1, 512:1024], kp1[:])
                sq = smp.tile([128, C], f32, tag="sq", bufs=2, name=f"sq{s}")
                sk = smp.tile([128, C], f32, tag="sk", bufs=2, name=f"sk{s}")
                nc.scalar.square(sq[:], qc8[:, h2:h2 + 1, :])
                nc.scalar.square(sk[:], kc8[:, h2:h2 + 1, :])
                if s == 0:
                    nc.gpsimd.tensor_copy(accq[:], sq[:])
                    nc.gpsimd.tensor_copy(acck[:], sk[:])
                else:
                    nc.gpsimd.tensor_add(accq[:], accq[:], sq[:])
                    nc.gpsimd.tensor_add(acck[:], acck[:], sk[:])
                if h2 == 1:
                    for h in range(H):
                        for m in range(2):
                            off = h * 256 + m * 128
                            nc.tensor.matmul(
                                st_slice(h, m),
                                kc8[:, :, off:off + 128],
                                qc8[:, :, h * 256:(h + 1) * 256],
                                start=(s == 1), stop=(s == 15),
                                perf_mode=DR, skip_group_check=True)

            # sumsq rows: [1, 512] ones-matmuls into the freed qk psum slots
            accq16 = smp.tile([128, C], bf16, tag="accq16", name="accq16")
            acck16 = smp.tile([128, C], bf16, tag="acck16", name="acck16")
            nc.vector.tensor_copy(accq16[:], accq[:])
            nc.vector.tensor_copy(acck16[:], acck[:])
            ss_ps = []
            for i, (src, lo) in enumerate([(accq16, 0), (accq16, 512),
                                           (acck16, 0), (acck16, 512)]):
                sp = ps1.tile([1, 512], f32, tag=P1[i], name=f"ss{i}")
                nc.tensor.matmul(sp[:], ones_sb[:], src[:, lo:lo + 512],
                                 start=True, stop=True)
                ss_ps.append(sp)

            # bf16 SBUF bounces for the collective input
            stA_sb = strp.tile([128, 1024], bf16, name="stA_sb")
            stB_sb = strp.tile([128, 1024], bf16, name="stB_sb")
            nc.vector.tensor_copy(stA_sb[:], stA[:])
            nc.vector.tensor_copy(stB_sb[:], stB[:])
            ss_sb = []
            for i in range(4):
                sb = strp.tile([1, 512], bf16, name=f"ssb{i}")
                nc.vector.tensor_copy(sb[:], ss_ps[i][:])
                ss_sb.append(sb)

            # ---------------- AllReduce over batch-pairs (bf16) -----------
            CCN = 128 * 2048 + 2 * C
            cc_in = dramp.tile([CCN], bf16, name="cc_in")
            cc_out = dramp.tile([CCN], bf16, name="cc_out")
            nc.sync.dma_start(
                cc_in[0:131072].rearrange("(p f) -> p f", p=128), stA_sb[:])
            nc.sync.dma_start(
                cc_in[131072:262144].rearrange("(p f) -> p f", p=128), stB_sb[:])
            for i in range(4):
                nc.sync.dma_start(
                    cc_in[262144 + i * 512: 262144 + (i + 1) * 512]
                    .rearrange("(a f) -> a f", a=1), ss_sb[i][:])
            nc.gpsimd.collective_compute(
                "AllReduce", ADD,
                replica_groups=[[0, 1], [2, 3], [4, 5], [6, 7]],
                ins=[cc_in.opt()], outs=[cc_out.opt()])
            strA = strp.tile([128, 1024], bf16, name="strA")
            strB = strp.tile([128, 1024], bf16, name="strB")
            nc.sync.dma_start(
                strA[:], cc_out[0:131072].rearrange("(p f) -> p f", p=128))
            nc.sync.dma_start(
                strB[:], cc_out[131072:262144].rearrange("(p f) -> p f", p=128))
            ssred = constp.tile([128, 16], bf16, name="ssred")
            nc.sync.dma_start(
                ssred[:],
                cc_out[262144:262144 + 2048].rearrange("(j p) -> p j", p=128))

            def str_slice(h, m):
                t = strA if h < 2 else strB
                off = (h % 2) * 512 + m * 256
                return t[:, off:off + 256]

            # ---------------- phase A2: v (overlaps the collective) -------
            wv = []
            for t in range(T4):
                w = wgtp.tile([128, 2, C], f8, tag=f"wq{t}", name=f"wv{t}")
                nc.sync.dma_start(w[:], wv_d[t])
                wv.append(w)
            vh = [vop.tile([128, 2, NL], f8, tag=f"vh{h}", name=f"vh{h}")
                  for h in range(H)]
            for cv in range(8):
                vps = [ps1.tile([128, 512], f32, tag=P1[w], name=f"vp{cv}_{w}")
                       for w in range(4)]
                for t in range(T4):
                    fl, ll = (t == 0), (t == T4 - 1)
                    lhs = wv[t][:, :, cv * 128:(cv + 1) * 128]
                    for w in range(4):
                        nc.tensor.matmul(vps[w][:], lhs,
                                         xall[t][:, :, w * 512:(w + 1) * 512],
                                         start=fl, stop=ll, perf_mode=DR)
                for w in range(4):
                    nc.vector.tensor_scalar_mul(
                        vh[cv // 2][:, cv % 2:cv % 2 + 1,
                                    w * 512:(w + 1) * 512], vps[w][:], 0.5)

            # proj weights (overlap collective tail)
            wp = []
            for t in range(T4):
                w = wgtp.tile([128, 2, C], f8, tag=f"wk{t}", name=f"wp{t}")
                nc.sync.dma_start(w[:], wp_d[t])
                wp.append(w)

            # ---------------- phase B: normalize + softmax + attn@v -------
            # rq = temp/max(sqrt(ssq),eps), rk likewise, per-channel-chunk
            # columns [128, 16]: cols 0-7 = rq, 8-15 = rk.
            rqk = constp.tile([128, 16], f32, name="rqk")
            nc.scalar.sqrt(rqk[:], ssred[:])
            nc.vector.tensor_scalar_max(rqk[:], rqk[:], EPS)
            nc.vector.reciprocal(rqk[:], rqk[:])
            nc.vector.tensor_mul(rqk[:, 0:8], rqk[:, 0:8], tmpv_sb[:])

            otp = [vop.tile([128, 2, NL], f8, tag=f"ot{h}", name=f"ot{h}")
                   for h in range(H)]
            for h in range(H):
                # S[c,d] = rq[c]*rk[d]*G^T[d,c]; rk applied via diag-matmul
                dgs = []
                for md in range(2):
                    dg = bp.tile([128, 128], bf16, tag=f"dg{md}", bufs=2,
                                 name=f"dg{h}_{md}")
                    nc.vector.tensor_scalar_mul(
                        dg[:], ident[:], rqk[:, 8 + 2 * h + md:9 + 2 * h + md])
                    dgs.append(dg)
                spm = ps1.tile([128, 512], f32, tag="pA", name=f"spm{h}")
                for mc in range(2):
                    for md in range(2):
                        base = (h % 2) * 512 + md * 256 + mc * 128
                        src = (strA if h < 2 else strB)[:, base:base + 128]
                        nc.tensor.matmul(
                            spm[:, mc * 256 + md * 128:
                                mc * 256 + (md + 1) * 128],
                            src, dgs[md][:], start=True, stop=True)
                # softmax over d (free axis); rq folded into Exp scale
                negmax = bp.tile([128, 2], f32, tag="negmax", bufs=2,
                                 name=f"negmax{h}")
                nmq = bp.tile([128, 2], f32, tag="nmq", bufs=2, name=f"nmq{h}")
                rowsum = bp.tile([128, 2], f32, tag="rowsum", bufs=2,
                                 name=f"rowsum{h}")
                recip = bp.tile([128, 2], f32, tag="recip", bufs=2,
                                name=f"recip{h}")
                esb = bp.tile([128, 512], f32, tag="esb", bufs=2,
                              name=f"esb{h}")
                for mc in range(2):
                    nc.vector.reduce_max(negmax[:, mc:mc + 1],
                                         spm[:, mc * 256:(mc + 1) * 256],
                                         axis=AX, negate=True)
                    nc.vector.tensor_mul(nmq[:, mc:mc + 1],
                                         negmax[:, mc:mc + 1],
                                         rqk[:, 2 * h + mc:2 * h + mc + 1])
                    nc.scalar.activation(esb[:, mc * 256:(mc + 1) * 256],
                                         spm[:, mc * 256:(mc + 1) * 256],
                                         Exp, bias=nmq[:, mc:mc + 1],
                                         scale=rqk[:, 2 * h + mc:2 * h + mc + 1],
                                         accum_out=rowsum[:, mc:mc + 1])
                nc.vector.reciprocal(recip[:], rowsum[:])
                nc.vector.tensor_scalar_mul(recip[:], recip[:], 1.0 / 16.0)
                # attn^T as fp8 (x64), 2-slab layout for DoubleRow
                atp = ps1.tile([128, 512], f32, tag="pB", name=f"atp{h}")
                for md in range(2):
                    for mc in range(2):
                        nc.tensor.transpose(
                            atp[:, md * 256 + mc * 128:
                                md * 256 + (mc + 1) * 128],
                            esb[:, mc * 256 + md * 128:
                                mc * 256 + (md + 1) * 128],
                            ident[:])
                atn = bp.tile([128, 2, 256], f8, tag="atn", bufs=2,
                              name=f"atn{h}")
                for md in range(2):
                    nc.vector.tensor_scalar_mul(
                        atn[:, md:md + 1, :],
                        atp[:, md * 256:(md + 1) * 256], 64.0)
                # out^T[c,:] = sum_d attn^T[d,c] v8[d,:], scaled 1/(16*rowsum)
                for mc in range(2):
                    for nfh in range(2):
                        op2 = ps2.tile([128, 1024], f32,
                                       tag=("pE" if nfh == 0 else "pF"),
                                       name=f"op2_{h}_{mc}_{nfh}")
                        for n2 in range(2):
                            nc.tensor.matmul(
                                op2[:, n2 * 512:(n2 + 1) * 512],
                                atn[:, :, mc * 128:(mc + 1) * 128],
                                vh[h][:, :, nfh * 1024 + n2 * 512:
                                      nfh * 1024 + (n2 + 1) * 512],
                                start=True, stop=True, perf_mode=DR)
                        nc.vector.tensor_scalar_mul(
                            otp[h][:, mc:mc + 1, nfh * 1024:(nfh + 1) * 1024],
                            op2[:], recip[:, mc:mc + 1])

            # ---------------- phase C: projection + bias + residual -------
            for j in range(8):
                pq = [ps1.tile([128, 512], f32, tag=P1[q], name=f"pp{j}_{q}")
                      for q in range(4)]
                for r in range(4):
                    lhs = wp[r][:, :, j * 128:(j + 1) * 128]
                    for q in range(4):
                        nc.tensor.matmul(
                            pq[q][:], lhs,
                            otp[q][:, :, r * 512:(r + 1) * 512],
                            start=(r == 0), stop=(r == 3), perf_mode=DR)
                for q in range(4):
                    xr = cp.tile([128, 512], f32, tag=f"xr{q}", bufs=2,
                                 name=f"xr{j}_{q}")
                    nc.sync.dma_start(
                        xr[:], xrT_d[j * 128:(j + 1) * 128,
                                     q * 512:(q + 1) * 512])
                    yq = cp.tile([128, 512], f32, tag=f"yq{q}", bufs=2,
                                 name=f"yq{j}_{q}")
                    nc.scalar.activation(yq[:], pq[q][:], Ident,
                                         bias=bias_sb[:, j:j + 1],
                                         scale=1.0 / 2048.0)
                    nc.vector.tensor_add(yq[:], yq[:], xr[:])
                    nc.sync.dma_start(
                        yT_d[j * 128:(j + 1) * 128, q * 512:(q + 1) * 512],
                        yq[:])

    nc.compile()
    return nc


def _get_nc():
    if "nc" not in _CACHE:
        _CACHE["nc"] = _build()
    return _CACHE["nc"]


def _to_fp8(a):
    import ml_dtypes
    return np.clip(a, -240.0, 240.0).astype(ml_dtypes.float8_e4m3)


def _swz(aT):
    """[C, M] -> fp8 [T4, 128, 2, M]: row t*256 + i*128 + p -> [t, p, i]."""
    C_, M = aT.shape
    r = _to_fp8(aT).reshape(T4, 2, 128, M)
    return np.ascontiguousarray(r.transpose(0, 2, 1, 3))


def _make_in_maps(x, Wqkv, Wproj, bproj, temperature):
    x = np.ascontiguousarray(np.asarray(x, dtype=np.float32))
    Wqkv = np.asarray(Wqkv, dtype=np.float32)
    Wproj = np.asarray(Wproj, dtype=np.float32)
    bproj = np.asarray(bproj, dtype=np.float32).reshape(C)
    temp = np.asarray(temperature, dtype=np.float32).reshape(H)

    WqkvT = Wqkv.T  # [C, 3C]
    wqk8 = _swz(32.0 * WqkvT[:, :2 * C])
    wv8 = _swz(32.0 * WqkvT[:, 2 * C:])
    wp8 = _swz(32.0 * Wproj.T)
    bias2d = np.ascontiguousarray(bproj.reshape(8, 128).T)
    tmpv2d = np.ascontiguousarray(np.repeat(temp, HD).reshape(8, 128).T)

    # residue-grouped local token order: g = (n%4)*512 + n//4
    perm = np.concatenate([np.arange(r, NL, 4) for r in range(4)])

    in_maps = []
    for core in range(NCORES):
        b, half = core // 2, core % 2
        xT = x[b, half * NL:(half + 1) * NL, :].T        # [C, NL]
        xs8 = _swz(xT[:, perm])
        rows = _out_rows(half)
        xrT = np.ascontiguousarray(x[b, rows, :].T)
        in_maps.append(dict(xs=xs8, xrT=xrT, wqk=wqk8, wv=wv8, wp=wp8,
                            bias=bias2d, tmpv=tmpv2d))
    return in_maps


def _out_rows(half):
    # torch transpose+reshape scramble: this core's y rows
    return np.concatenate(
        [h * 1024 + half * 512 + np.arange(512) for h in range(H)])


def _run(in_maps, trace=False, **kw):
    from concourse.bass_utils import run_bass_kernel_spmd

    nc = _get_nc()
    return run_bass_kernel_spmd(nc, in_maps, core_ids=list(range(NCORES)),
                                trace=trace, **kw)


def kernel(x, Wqkv, Wproj, bproj, temperature):
    res = _run(_make_in_maps(x, Wqkv, Wproj, bproj, temperature))
    y = np.empty((B, N, C), dtype=np.float32)
    for core in range(NCORES):
        b, half = core // 2, core % 2
        y[b, _out_rows(half), :] = res.results[core]["yT"].T
    return y


# revision 13
# speedup vs baseline: 1.9032x; 1.1312x over previous
"""Efficient Channel Attention kernel for 8 Trainium2 NeuronCores.

Problem (B=4, N=4096, C=1024, H=4, HD=256):
    qkv = x @ Wqkv.T                 -> q,k,v per head, [HD, N] layout
    q,k l2-normalized over N; scores = (q*temp) @ k.T   [HD, HD] per (b,h)
    attn = softmax(scores, -1); out = attn @ v; y = out @ Wproj.T + bproj + x

Sharding: core = (batch b, token-half). All channel contractions are local;
the only cross-core coupling is the token(N)-contracted quantities: the raw
Grams k^T q and the q/k squared norms, AllReduce'd (bf16, ~0.5MB) within
the core pair sharing a batch.

All large GEMMs (qkv, v, Gram, attn@v, proj) run in fp8e4m3 with the
DoubleRow perf mode (2 k-slabs of 128 per instruction). Scales are chosen
so fp8 operands sit in the format's sweet spot and cancel exactly through
the l2 normalization / softmax math:
    x8 = fp8(x); w8 = fp8(32*W)  ->  q,k,v come out 32x
    v8 = fp8(0.5 * v_psum) = 16*v ; atn8 = fp8(64 * exp)
    out8 = fp8(op2 / (16*rowsum)) = 64*out ; proj psum = 2048*y -> scale 1/2048

Tokens are processed in "residue-grouped" order g = (n%4)*512 + n//4 (host
permutes x once). A1/Gram/sumsq are order-invariant; v and out^T then come
out grouped so the proj GEMM's moving operand is fully contiguous and the
torch transpose+reshape scramble costs nothing.
"""

import numpy as np

B, N, C, H = 4, 4096, 1024, 4
HD = C // H          # 256
NCORES = 8
NL = N // 2          # 2048 tokens per core
T4 = C // 256        # 4 channel super-chunks (2 slabs of 128 each)
EPS = 1e-12

_CACHE = {}


def _build():
    import concourse.mybir as mybir
    import concourse.tile as tile
    from concourse import bacc
    from concourse.masks import make_identity

    f32 = mybir.dt.float32
    f32r = mybir.dt.float32r
    bf16 = mybir.dt.bfloat16
    f8 = mybir.dt.float8e4
    DR = mybir.MatmulPerfMode.DoubleRow
    AX = mybir.AxisListType.X
    ADD = mybir.AluOpType.add
    Exp = mybir.ActivationFunctionType.Exp
    Ident = mybir.ActivationFunctionType.Identity

    nc = bacc.Bacc("TRN2", target_bir_lowering=False, debug=False,
                   num_devices=NCORES)

    xs_d = nc.dram_tensor("xs", [T4, 128, 2, NL], f8, kind="ExternalInput").ap()
    wqk_d = nc.dram_tensor("wqk", [T4, 128, 2, 2 * C], f8,
                           kind="ExternalInput").ap()
    wv_d = nc.dram_tensor("wv", [T4, 128, 2, C], f8, kind="ExternalInput").ap()
    wp_d = nc.dram_tensor("wp", [T4, 128, 2, C], f8, kind="ExternalInput").ap()
    bias_d = nc.dram_tensor("bias", [128, 8], f32, kind="ExternalInput").ap()
    tmpv_d = nc.dram_tensor("tmpv", [128, 8], f32, kind="ExternalInput").ap()
    xrT_d = nc.dram_tensor("xrT", [C, NL], f32, kind="ExternalInput").ap()
    yT_d = nc.dram_tensor("yT", [C, NL], f32, kind="ExternalOutput").ap()

    with tile.TileContext(nc) as tc:
        with (
            tc.tile_pool(name="const", bufs=1) as constp,
            tc.tile_pool(name="wgt", bufs=1) as wgtp,
            tc.tile_pool(name="xst", bufs=1) as xsp,
            tc.tile_pool(name="xall", bufs=1) as xap,
            tc.tile_pool(name="qk", bufs=1) as qkp,
            tc.tile_pool(name="sums", bufs=1) as smp,
            tc.tile_pool(name="vo", bufs=1) as vop,
            tc.tile_pool(name="str", bufs=1) as strp,
            tc.tile_pool(name="bph", bufs=1) as bp,
            tc.tile_pool(name="cph", bufs=1) as cp,
            tc.tile_pool(name="ps1", bufs=1, space="PSUM") as ps1,
            tc.tile_pool(name="ps2", bufs=1, space="PSUM") as ps2,
            tc.tile_pool(name="dram", bufs=1, space="DRAM") as dramp,
        ):
            P1 = ["pA", "pB", "pC", "pD"]  # 1-bank rotating psum tags

            # ---------------- constants ----------------
            ident = constp.tile([128, 128], f32, name="ident")
            make_identity(nc, ident[:])
            bias_sb = constp.tile([128, 8], f32, name="bias_sb")
            nc.sync.dma_start(bias_sb[:], bias_d[:])
            tmpv_sb = constp.tile([128, 8], f32, name="tmpv_sb")
            nc.sync.dma_start(tmpv_sb[:], tmpv_d[:])
            ones_sb = constp.tile([128, 1], bf16, name="ones_sb")
            nc.vector.memset(ones_sb[:], 1.0)

            # qk weights + first x super-tile ahead of everything else
            wqkq, wqkk = [], []
            xst0 = []
            for t in range(T4):
                wq = wgtp.tile([128, 2, C], f8, tag=f"wq{t}", name=f"wqkq{t}")
                nc.sync.dma_start(wq[:], wqk_d[t, :, :, 0:C])
                wqkq.append(wq)
                wk = wgtp.tile([128, 2, C], f8, tag=f"wk{t}", name=f"wqkk{t}")
                nc.sync.dma_start(wk[:], wqk_d[t, :, :, C:2 * C])
                wqkk.append(wk)
                xt = xsp.tile([128, 2, 512], f8, tag=f"xs{t}", bufs=2,
                              name=f"xa{t}_0")
                nc.sync.dma_start(xt[:], xs_d[t, :, :, 0:512])
                xst0.append(xt)

            # Gram accumulators: stA = heads 0,1 / stB = heads 2,3
            stA = ps2.tile([128, 1024], f32, tag="pE", name="stA")
            stB = ps2.tile([128, 1024], f32, tag="pF", name="stB")

            def st_slice(h, m):
                t = stA if h < 2 else stB
                off = (h % 2) * 512 + m * 256
                return t[:, off:off + 256]

            accq = smp.tile([128, C], f32, tag="accq", name="accq")
            acck = smp.tile([128, C], f32, tag="acck", name="acck")

            # ---------------- phase A1: q,k + Grams + sumsq ----------------
            qc8 = kc8 = None
            xall = []
            xst = xst0
            for s in range(16):
                n5, sub = s // 4, s % 4
                if n5 > 0 and sub == 0:
                    xst = []
                    for t in range(T4):
                        xt = xsp.tile([128, 2, 512], f8, tag=f"xs{t}", bufs=2,
                                      name=f"xa{t}_{n5}")
                        nc.sync.dma_start(
                            xt[:], xs_d[t, :, :, n5 * 512:(n5 + 1) * 512])
                        xst.append(xt)
                if s == 5:
                    # full x resident for phase A2 (v); issued here so the
                    # DMA queue drains A1's startup tiles first
                    for t in range(T4):
                        xt = xap.tile([128, 2, NL], f8, tag=f"xf{t}",
                                      name=f"xall{t}")
                        nc.sync.dma_start(xt[:], xs_d[t])
                        xall.append(xt)
                    # v / proj weights: own tags so these overlap A1 too
                    wv, wp = [], []
                    for t in range(T4):
                        w = wgtp.tile([128, 2, C], f8, tag=f"wv{t}",
                                      name=f"wv{t}")
                        nc.sync.dma_start(w[:], wv_d[t])
                        wv.append(w)
                    for t in range(T4):
                        w = wgtp.tile([128, 2, C], f8, tag=f"wp{t}",
                                      name=f"wp{t}")
                        nc.sync.dma_start(w[:], wp_d[t])
                        wp.append(w)
                h2 = s % 2
                if h2 == 0:
                    qc8 = qkp.tile([128, 2, C], f8, tag="qc8", bufs=2,
                                   name=f"qc8_{s // 2}")
                    kc8 = qkp.tile([128, 2, C], f8, tag="kc8", bufs=2,
                                   name=f"kc8_{s // 2}")
                qp0 = ps1.tile([128, 512], f32, tag="pA", name="qp0")
                qp1 = ps1.tile([128, 512], f32, tag="pB", name="qp1")
                kp0 = ps1.tile([128, 512], f32, tag="pC", name="kp0")
                kp1 = ps1.tile([128, 512], f32, tag="pD", name="kp1")
                for t in range(T4):
                    lhs = xst[t][:, :, sub * 128:(sub + 1) * 128]
                    fl, ll = (t == 0), (t == T4 - 1)
                    nc.tensor.matmul(qp0[:], lhs, wqkq[t][:, :, 0:512],
                                     start=fl, stop=ll, perf_mode=DR)
                    nc.tensor.matmul(qp1[:], lhs, wqkq[t][:, :, 512:1024],
                                     start=fl, stop=ll, perf_mode=DR)
                    nc.tensor.matmul(kp0[:], lhs, wqkk[t][:, :, 0:512],
                                     start=fl, stop=ll, perf_mode=DR)
                    nc.tensor.matmul(kp1[:], lhs, wqkk[t][:, :, 512:1024],
                                     start=fl, stop=ll, perf_mode=DR)
                nc.vector.tensor_copy(qc8[:, h2:h2 + 1, 0:512], qp0[:])
                nc.vector.tensor_copy(qc8[:, h2:h2 + 1, 512:1024], qp1[:])
                nc.vector.tensor_copy(kc8[:, h2:h2 + 1, 0:512], kp0[:])
                nc.vector.tensor_copy(kc8[:, h2:h2 + 1, 512:1024], kp1[:])
                sq = smp.tile([128, C], f32, tag="sq", bufs=2, name=f"sq{s}")
                sk = smp.tile([128, C], f32, tag="sk", bufs=2, name=f"sk{s}")
                nc.scalar.square(sq[:], qc8[:, h2:h2 + 1, :])
                nc.scalar.square(sk[:], kc8[:, h2:h2 + 1, :])
                # accumulate chains split across engines so each keeps pace
                # with the PE block rate (gpsimd alone fell ~20us behind)
                if s == 0:
                    nc.vector.tensor_copy(accq[:], sq[:])
                    nc.gpsimd.tensor_copy(acck[:], sk[:])
                else:
                    nc.vector.tensor_add(accq[:], accq[:], sq[:])
                    nc.gpsimd.tensor_add(acck[:], acck[:], sk[:])
                if h2 == 1:
                    for h in range(H):
                        for m in range(2):
                            off = h * 256 + m * 128
                            nc.tensor.matmul(
                                st_slice(h, m),
                                kc8[:, :, off:off + 128],
                                qc8[:, :, h * 256:(h + 1) * 256],
                                start=(s == 1), stop=(s == 15),
                                perf_mode=DR, skip_group_check=True)

            # sumsq rows: [1, 512] ones-matmuls into the freed qk psum slots
            accq16 = smp.tile([128, C], bf16, tag="accq16", name="accq16")
            acck16 = smp.tile([128, C], bf16, tag="acck16", name="acck16")
            nc.vector.tensor_copy(accq16[:], accq[:])
            nc.vector.tensor_copy(acck16[:], acck[:])
            ss_ps = []
            for i, (src, lo) in enumerate([(accq16, 0), (accq16, 512),
                                           (acck16, 0), (acck16, 512)]):
                sp = ps1.tile([1, 512], f32, tag=P1[i], name=f"ss{i}")
                nc.tensor.matmul(sp[:], ones_sb[:], src[:, lo:lo + 512],
                                 start=True, stop=True)
                ss_ps.append(sp)

            # bf16 SBUF bounces for the collective input
            stA_sb = strp.tile([128, 1024], bf16, name="stA_sb")
            stB_sb = strp.tile([128, 1024], bf16, name="stB_sb")
            nc.vector.tensor_copy(stA_sb[:], stA[:])
            nc.vector.tensor_copy(stB_sb[:], stB[:])
            ss_sb = []
            for i in range(4):
                sb = strp.tile([1, 512], bf16, name=f"ssb{i}")
                nc.vector.tensor_copy(sb[:], ss_ps[i][:])
                ss_sb.append(sb)

            # ---------------- AllReduce over batch-pairs (bf16) -----------
            CCN = 128 * 2048 + 2 * C
            cc_in = dramp.tile([CCN], bf16, name="cc_in")
            cc_out = dramp.tile([CCN], bf16, name="cc_out")
            nc.sync.dma_start(
                cc_in[0:131072].rearrange("(p f) -> p f", p=128), stA_sb[:])
            nc.sync.dma_start(
                cc_in[131072:262144].rearrange("(p f) -> p f", p=128), stB_sb[:])
            for i in range(4):
                nc.sync.dma_start(
                    cc_in[262144 + i * 512: 262144 + (i + 1) * 512]
                    .rearrange("(a f) -> a f", a=1), ss_sb[i][:])
            nc.gpsimd.collective_compute(
                "AllReduce", ADD,
                replica_groups=[[0, 1], [2, 3], [4, 5], [6, 7]],
                ins=[cc_in.opt()], outs=[cc_out.opt()])
            strA = strp.tile([128, 1024], bf16, name="strA")
            strB = strp.tile([128, 1024], bf16, name="strB")
            nc.sync.dma_start(
                strA[:], cc_out[0:131072].rearrange("(p f) -> p f", p=128))
            nc.sync.dma_start(
                strB[:], cc_out[131072:262144].rearrange("(p f) -> p f", p=128))
            ssred = constp.tile([128, 16], bf16, name="ssred")
            nc.sync.dma_start(
                ssred[:],
                cc_out[262144:262144 + 2048].rearrange("(j p) -> p j", p=128))

            def str_slice(h, m):
                t = strA if h < 2 else strB
                off = (h % 2) * 512 + m * 256
                return t[:, off:off + 256]

            # ---------------- phase A2: v (overlaps the collective) -------
            vh = [vop.tile([128, 2, NL], f8, tag=f"vh{h}", name=f"vh{h}")
                  for h in range(H)]
            for cv in range(8):
                vps = [ps1.tile([128, 512], f32, tag=P1[w], name=f"vp{cv}_{w}")
                       for w in range(4)]
                for t in range(T4):
                    fl, ll = (t == 0), (t == T4 - 1)
                    lhs = wv[t][:, :, cv * 128:(cv + 1) * 128]
                    for w in range(4):
                        nc.tensor.matmul(vps[w][:], lhs,
                                         xall[t][:, :, w * 512:(w + 1) * 512],
                                         start=fl, stop=ll, perf_mode=DR)
                for w in range(4):
                    nc.vector.tensor_scalar_mul(
                        vh[cv // 2][:, cv % 2:cv % 2 + 1,
                                    w * 512:(w + 1) * 512], vps[w][:], 0.5)

            # ---------------- phase B: normalize + softmax + attn@v -------
            # rq = temp/max(sqrt(ssq),eps), rk likewise, per-channel-chunk
            # columns [128, 16]: cols 0-7 = rq, 8-15 = rk.
            rqk = constp.tile([128, 16], f32, name="rqk")
            nc.scalar.sqrt(rqk[:], ssred[:])
            nc.vector.tensor_scalar_max(rqk[:], rqk[:], EPS)
            nc.vector.reciprocal(rqk[:], rqk[:])
            nc.vector.tensor_mul(rqk[:, 0:8], rqk[:, 0:8], tmpv_sb[:])

            otp = [vop.tile([128, 2, NL], f8, tag=f"ot{h}", name=f"ot{h}")
                   for h in range(H)]

            def phase_b(h):
                # S[c,d] = rq[c]*rk[d]*G^T[d,c]; rk applied via diag-matmul
                dgs = []
                for md in range(2):
                    dg = bp.tile([128, 128], bf16, tag=f"dg{md}", bufs=2,
                                 name=f"dg{h}_{md}")
                    nc.vector.tensor_scalar_mul(
                        dg[:], ident[:], rqk[:, 8 + 2 * h + md:9 + 2 * h + md])
                    dgs.append(dg)
                spm = ps1.tile([128, 512], f32, tag="pA", name=f"spm{h}")
                for mc in range(2):
                    for md in range(2):
                        base = (h % 2) * 512 + md * 256 + mc * 128
                        src = (strA if h < 2 else strB)[:, base:base + 128]
                        nc.tensor.matmul(
                            spm[:, mc * 256 + md * 128:
                                mc * 256 + (md + 1) * 128],
                            src, dgs[md][:], start=True, stop=True)
                # softmax over d (free axis); rq folded into Exp scale
                negmax = bp.tile([128, 2], f32, tag="negmax", bufs=2,
                                 name=f"negmax{h}")
                nmq = bp.tile([128, 2], f32, tag="nmq", bufs=2, name=f"nmq{h}")
                rowsum = bp.tile([128, 2], f32, tag="rowsum", bufs=2,
                                 name=f"rowsum{h}")
                recip = bp.tile([128, 2], f32, tag="recip", bufs=2,
                                name=f"recip{h}")
                esb = bp.tile([128, 512], f32, tag="esb", bufs=2,
                              name=f"esb{h}")
                for mc in range(2):
                    nc.vector.reduce_max(negmax[:, mc:mc + 1],
                                         spm[:, mc * 256:(mc + 1) * 256],
                                         axis=AX, negate=True)
                    nc.vector.tensor_mul(nmq[:, mc:mc + 1],
                                         negmax[:, mc:mc + 1],
                                         rqk[:, 2 * h + mc:2 * h + mc + 1])
                    nc.scalar.activation(esb[:, mc * 256:(mc + 1) * 256],
                                         spm[:, mc * 256:(mc + 1) * 256],
                                         Exp, bias=nmq[:, mc:mc + 1],
                                         scale=rqk[:, 2 * h + mc:2 * h + mc + 1],
                                         accum_out=rowsum[:, mc:mc + 1])
                nc.vector.reciprocal(recip[:], rowsum[:])
                nc.vector.tensor_scalar_mul(recip[:], recip[:], 1.0 / 16.0)
                # attn^T as fp8 (x64), 2-slab layout for DoubleRow
                atp = ps1.tile([128, 512], f32, tag="pB", name=f"atp{h}")
                for md in range(2):
                    for mc in range(2):
                        nc.tensor.transpose(
                            atp[:, md * 256 + mc * 128:
                                md * 256 + (mc + 1) * 128],
                            esb[:, mc * 256 + md * 128:
                                mc * 256 + (md + 1) * 128],
                            ident[:])
                atn = bp.tile([128, 2, 256], f8, tag="atn", bufs=2,
                              name=f"atn{h}")
                for md in range(2):
                    nc.vector.tensor_scalar_mul(
                        atn[:, md:md + 1, :],
                        atp[:, md * 256:(md + 1) * 256], 64.0)
                # out^T[c,:] = sum_d attn^T[d,c] v8[d,:], scaled 1/(16*rowsum)
                for mc in range(2):
                    for nfh in range(2):
                        op2 = ps2.tile([128, 1024], f32,
                                       tag=("pE" if nfh == 0 else "pF"),
                                       name=f"op2_{h}_{mc}_{nfh}")
                        for n2 in range(2):
                            nc.tensor.matmul(
                                op2[:, n2 * 512:(n2 + 1) * 512],
                                atn[:, :, mc * 128:(mc + 1) * 128],
                                vh[h][:, :, nfh * 1024 + n2 * 512:
                                      nfh * 1024 + (n2 + 1) * 512],
                                start=True, stop=True, perf_mode=DR)
                        nc.vector.tensor_scalar_mul(
                            otp[h][:, mc:mc + 1, nfh * 1024:(nfh + 1) * 1024],
                            op2[:], recip[:, mc:mc + 1])

            # ---------------- phase C: projection + bias + residual -------
            # run per head-pair, interleaved with phase B, so the xr/y DMA
            # spreads across B's compute instead of draining at the end.
            # Uses psum tags pC/pD which B does not touch.
            def phase_c(hp):
                for j in range(8):
                    pq = {}
                    for qi, q in enumerate((2 * hp, 2 * hp + 1)):
                        pq[q] = ps1.tile([128, 512], f32, tag=P1[2 + qi],
                                         name=f"pp{hp}_{j}_{q}")
                    for r in range(4):
                        lhs = wp[r][:, :, j * 128:(j + 1) * 128]
                        for q in (2 * hp, 2 * hp + 1):
                            nc.tensor.matmul(
                                pq[q][:], lhs,
                                otp[q][:, :, r * 512:(r + 1) * 512],
                                start=(r == 0), stop=(r == 3), perf_mode=DR)
                    for qi, q in enumerate((2 * hp, 2 * hp + 1)):
                        xr = cp.tile([128, 512], f32, tag=f"xr{qi}", bufs=2,
                                     name=f"xr{j}_{q}")
                        nc.sync.dma_start(
                            xr[:], xrT_d[j * 128:(j + 1) * 128,
                                         q * 512:(q + 1) * 512])
                        yq = cp.tile([128, 512], f32, tag=f"yq{qi}", bufs=2,
                                     name=f"yq{j}_{q}")
                        nc.scalar.activation(yq[:], pq[q][:], Ident,
                                             bias=bias_sb[:, j:j + 1],
                                             scale=1.0 / 2048.0)
                        nc.vector.tensor_add(yq[:], yq[:], xr[:])
                        nc.sync.dma_start(
                            yT_d[j * 128:(j + 1) * 128,
                                 q * 512:(q + 1) * 512], yq[:])

            for hp in range(2):
                phase_b(2 * hp)
                phase_b(2 * hp + 1)
                phase_c(hp)

    nc.compile()
    return nc


def _get_nc():
    if "nc" not in _CACHE:
        _CACHE["nc"] = _build()
    return _CACHE["nc"]


def _to_fp8(a):
    import ml_dtypes
    return np.clip(a, -240.0, 240.0).astype(ml_dtypes.float8_e4m3)


def _swz(aT):
    """[C, M] -> fp8 [T4, 128, 2, M]: row t*256 + i*128 + p -> [t, p, i]."""
    C_, M = aT.shape
    r = _to_fp8(aT).reshape(T4, 2, 128, M)
    return np.ascontiguousarray(r.transpose(0, 2, 1, 3))


def _make_in_maps(x, Wqkv, Wproj, bproj, temperature):
    x = np.ascontiguousarray(np.asarray(x, dtype=np.float32))
    Wqkv = np.asarray(Wqkv, dtype=np.float32)
    Wproj = np.asarray(Wproj, dtype=np.float32)
    bproj = np.asarray(bproj, dtype=np.float32).reshape(C)
    temp = np.asarray(temperature, dtype=np.float32).reshape(H)

    WqkvT = Wqkv.T  # [C, 3C]
    wqk8 = _swz(32.0 * WqkvT[:, :2 * C])
    wv8 = _swz(32.0 * WqkvT[:, 2 * C:])
    wp8 = _swz(32.0 * Wproj.T)
    bias2d = np.ascontiguousarray(bproj.reshape(8, 128).T)
    tmpv2d = np.ascontiguousarray(np.repeat(temp, HD).reshape(8, 128).T)

    # residue-grouped local token order: g = (n%4)*512 + n//4
    perm = np.concatenate([np.arange(r, NL, 4) for r in range(4)])

    in_maps = []
    for core in range(NCORES):
        b, half = core // 2, core % 2
        xT = x[b, half * NL:(half + 1) * NL, :].T        # [C, NL]
        xs8 = _swz(xT[:, perm])
        rows = _out_rows(half)
        xrT = np.ascontiguousarray(x[b, rows, :].T)
        in_maps.append(dict(xs=xs8, xrT=xrT, wqk=wqk8, wv=wv8, wp=wp8,
                            bias=bias2d, tmpv=tmpv2d))
    return in_maps


def _out_rows(half):
    # torch transpose+reshape scramble: this core's y rows
    return np.concatenate(
        [h * 1024 + half * 512 + np.arange(512) for h in range(H)])


def _run(in_maps, trace=False, **kw):
    from concourse.bass_utils import run_bass_kernel_spmd

    nc = _get_nc()
    return run_bass_kernel_spmd(nc, in_maps, core_ids=list(range(NCORES)),
                                trace=trace, **kw)


def kernel(x, Wqkv, Wproj, bproj, temperature):
    res = _run(_make_in_maps(x, Wqkv, Wproj, bproj, temperature))
    y = np.empty((B, N, C), dtype=np.float32)
    for core in range(NCORES):
        b, half = core // 2, core % 2
        y[b, _out_rows(half), :] = res.results[core]["yT"].T
    return y
